# revision 5
# baseline (speedup 1.0000x reference)
"""GQA attention forward (B=4, T=1024, D=2048, 32 q-heads / 8 kv-heads, RoPE,
causal) distributed over 8 TRN2 NeuronCores.

Sharding: head-parallel tensor parallelism. Core c owns q-heads 4c..4c+3 and
kv-head c (wq/wk/wv column shards). Attention output (still sharded by head,
transposed layout [head_dim, tokens]) is re-sharded to token-parallel via one
AllToAll (2 MB/rank, bf16); each core then computes its 512-token row slice of
the output projection against the full wo.

Device layouts (per core):
  xT   [2048, 4096]  bf16  - x transposed, tokens batch-major
  qT   [128, 4096]x2 bf16  - 2 heads per tile, RoPE'd; head-dim de-interleaved
  kT2  [128, 4096]   bf16  - kv-head kT duplicated in both 64-partition halves
  vT   [64, 4096]    bf16  - PE-transposed per 128-token tile into v[128, 65]
                             with a ones column (softmax denominator trick)
  scores sT[k, q] in PSUM -> exp on ACT (scale=1/8 folded) -> bf16
  attn@v: lhsT = v_aug [128, 65], rhs = expT -> psum [65, 512] accumulated
  divide by denominator row via reciprocal + gpsimd partition_broadcast
  A2A -> aoT_g [2048(c), 512(t)] -> out[t, e] = sum_c aoT_g[c, t] * wo[c, e]

RoPE with de-interleaved head dims ([32 reals; 32 imags] per 64-row head):
  out = x*C + shift32(x*S), C = [c;c;...], S = [s;-s;s;-s] (host-built tiles).
"""

import sys

if "/opt/trn_rl_repo" not in sys.path:
    sys.path.insert(0, "/opt/trn_rl_repo")

import numpy as np
import ml_dtypes

import concourse.bass as bass
import concourse.mybir as mybir
import concourse.tile as tile
from concourse import bacc
from concourse.bass_utils import run_bass_kernel_spmd
from concourse.masks import make_identity, make_upper_triangular

BF16 = mybir.dt.bfloat16
F32 = mybir.dt.float32

B, T, D = 4, 1024, 2048
QH, KVH, HD = 32, 8, 64
N_CORES = 8
NT = B * T            # 4096 global tokens
NKO = D // 128        # 16 contraction subtiles
ROWS = NT // N_CORES  # 512 output rows per core
HPC = QH // N_CORES   # 4 q heads per core

_CACHE = {}


def _build():
    nc = bacc.Bacc("TRN2", target_bir_lowering=False, debug=False,
                   num_devices=N_CORES)

    xT = nc.dram_tensor("xT", [D, NT], BF16, kind="ExternalInput")
    wq = nc.dram_tensor("wq", [D, HPC * HD], BF16, kind="ExternalInput")
    wkv = nc.dram_tensor("wkv", [D, 2 * HD], BF16, kind="ExternalInput")
    wo = nc.dram_tensor("wo", [D, D], BF16, kind="ExternalInput")
    ct = nc.dram_tensor("ctile", [128, T], BF16, kind="ExternalInput")
    st = nc.dram_tensor("stile", [128, T], BF16, kind="ExternalInput")
    out = nc.dram_tensor("out", [ROWS, D], F32, kind="ExternalOutput")

    xT_r = xT.ap().rearrange("(ko p) t -> p ko t", p=128)
    wq_r = wq.ap().rearrange("(ko p) m -> p ko m", p=128)
    wkv_r = wkv.ap().rearrange("(ko p) m -> p ko m", p=128)
    wo_r = wo.ap().rearrange("(ko p) e -> p ko e", p=128)

    with tile.TileContext(nc) as tc:
        import contextlib
        with contextlib.ExitStack() as ctx:
            const = ctx.enter_context(tc.tile_pool(name="const", bufs=1))
            xp = ctx.enter_context(tc.tile_pool(name="xp", bufs=3))
            big = ctx.enter_context(tc.tile_pool(name="big", bufs=1))
            vp = ctx.enter_context(tc.tile_pool(name="vp", bufs=2))
            ep = ctx.enter_context(tc.tile_pool(name="ep", bufs=4))
            xsp = ctx.enter_context(tc.tile_pool(name="xsp", bufs=2))
            dnp = ctx.enter_context(tc.tile_pool(name="dnp", bufs=3))
            bcp = ctx.enter_context(tc.tile_pool(name="bcp", bufs=2))
            wop = ctx.enter_context(tc.tile_pool(name="wop", bufs=2))
            gp = ctx.enter_context(tc.tile_pool(name="gp", bufs=1))
            op = ctx.enter_context(tc.tile_pool(name="op", bufs=2))
            dram = ctx.enter_context(tc.tile_pool(name="dram", bufs=1,
                                                  space="DRAM"))
            pp = ctx.enter_context(tc.tile_pool(name="pp", bufs=2,
                                                space="PSUM"))
            sp = ctx.enter_context(tc.tile_pool(name="sp", bufs=2,
                                                space="PSUM"))
            ap = ctx.enter_context(tc.tile_pool(name="ap", bufs=2,
                                                space="PSUM"))

            # constants / weights
            wq_sb = const.tile([128, NKO, HPC * HD], BF16, tag="wq")
            nc.sync.dma_start(wq_sb[:], wq_r)
            wkv_sb = const.tile([128, NKO, 2 * HD], BF16, tag="wkv")
            nc.sync.dma_start(wkv_sb[:], wkv_r)
            ct_sb = const.tile([128, T], BF16, tag="ct")
            nc.sync.dma_start(ct_sb[:], ct.ap())
            st_sb = const.tile([128, T], BF16, tag="st")
            nc.sync.dma_start(st_sb[:], st.ap())
            utri = const.tile([128, 128], BF16, tag="utri")
            make_upper_triangular(nc, utri[:], val=1.0, diag=True)
            ident = const.tile([64, 64], BF16, tag="ident")
            make_identity(nc, ident[:])

            qT = [big.tile([128, NT], BF16, tag=f"qT{hp}", name=f"qT{hp}")
                  for hp in range(2)]
            kT2 = big.tile([128, NT], BF16, tag="kT2")
            vT = big.tile([64, NT], BF16, tag="vT")
            aoT = [big.tile([128, NT], BF16, tag=f"aoT{hp}", name=f"aoT{hp}")
                   for hp in range(2)]

            def rope(dst, xs, xs2, ps, rows, cs_sl, ss_sl):
                # dst = ps*C + shift32(ps*S) over `rows` partitions (64 or 128)
                nc.vector.scalar_tensor_tensor(
                    dst, ps[0:rows], 1.0, cs_sl[0:rows],
                    mybir.AluOpType.mult, mybir.AluOpType.mult)
                nc.vector.scalar_tensor_tensor(
                    xs[0:rows], ps[0:rows], 1.0, ss_sl[0:rows],
                    mybir.AluOpType.mult, mybir.AluOpType.mult)
                # shift-by-32 within each 64-row half (cross-partition: gpsimd)
                for g in range(rows // 32):
                    a, b_ = g * 32, (g ^ 1) * 32
                    nc.gpsimd.tensor_copy(xs2[a:a + 32], xs[b_:b_ + 32])
                nc.vector.tensor_add(dst, dst, xs2[0:rows])

            for b in range(B):
                den_pack = dnp.tile([8, 512], F32, tag="den")
                rec_pack = dnp.tile([8, 512], F32, tag="rec")
                xc = []
                for half in range(2):
                    t0 = b * T + half * 512
                    xt = xp.tile([128, NKO, 512], BF16, tag="x")
                    nc.sync.dma_start(xt[:], xT_r[:, :, t0:t0 + 512])
                    xc.append(xt)

                # q projection + rope
                for hp in range(2):
                    for half in range(2):
                        t0 = b * T + half * 512
                        ps = pp.tile([128, 512], F32, tag="mm")
                        for ko in range(NKO):
                            nc.tensor.matmul(
                                ps[:], wq_sb[:, ko, hp * 128:hp * 128 + 128],
                                xc[half][:, ko, :],
                                start=(ko == 0), stop=(ko == NKO - 1))
                        xs = xsp.tile([128, 512], BF16, tag="xs")
                        xs2 = xsp.tile([128, 512], BF16, tag="xs2")
                        csl = ct_sb[:, half * 512:half * 512 + 512]
                        ssl = st_sb[:, half * 512:half * 512 + 512]
                        rope(qT[hp][:, t0:t0 + 512], xs, xs2, ps, 128,
                             csl, ssl)

                # kv projection: rope k into both halves of kT2; evac vT
                for half in range(2):
                    t0 = b * T + half * 512
                    ps = pp.tile([128, 512], F32, tag="mm")
                    for ko in range(NKO):
                        nc.tensor.matmul(
                            ps[:], wkv_sb[:, ko, :], xc[half][:, ko, :],
                            start=(ko == 0), stop=(ko == NKO - 1))
                    xs = xsp.tile([128, 512], BF16, tag="xs")
                    xs2 = xsp.tile([128, 512], BF16, tag="xs2")
                    csl = ct_sb[:, half * 512:half * 512 + 512]
                    ssl = st_sb[:, half * 512:half * 512 + 512]
                    rope(kT2[0:64, t0:t0 + 512], xs, xs2, ps, 64, csl, ssl)
                    nc.vector.tensor_copy(kT2[64:128, t0:t0 + 512],
                                          kT2[0:64, t0:t0 + 512])
                    nc.vector.tensor_copy(vT[:, t0:t0 + 512], ps[64:128, :])

                # transpose v into natural layout + ones column
                v_b = vp.tile([128, 8, HD + 1], BF16, tag="v")
                for j in range(8):
                    ks = b * T + j * 128
                    tps = pp.tile([128, 64], BF16, tag="mm")
                    nc.tensor.transpose(tps[:], vT[:, ks:ks + 128], ident[:])
                    nc.vector.tensor_copy(v_b[:, j, 0:HD], tps[:])
                nc.any.memset(v_b[:, :, HD:HD + 1], 1.0)

                # attention for this batch: 4 heads x 2 q-chunks of 512
                div_work = []
                for hp in range(2):
                    for h01 in range(2):
                        hbase = h01 * 64
                        for c in range(2):
                            po = ap.tile([65, 512], F32, tag="attn")
                            nj = 4 * c + 4
                            for j in range(nj):
                                q_lo = max(c * 512, j * 128)
                                N = (c + 1) * 512 - q_lo
                                q_rel = q_lo - c * 512
                                ks = b * T + j * 128
                                ps = sp.tile([128, 512], F32, tag="score")
                                nc.tensor.matmul(
                                    ps[:, :N],
                                    kT2[hbase:hbase + 64, ks:ks + 128],
                                    qT[hp][hbase:hbase + 64,
                                           b * T + q_lo:b * T + q_lo + N],
                                    start=True, stop=True)
                                et = ep.tile([128, 512], BF16, tag="et")
                                nc.scalar.activation(
                                    et[:, :N], ps[:, :N],
                                    mybir.ActivationFunctionType.Exp,
                                    scale=0.125)
                                if j >= 4 * c:
                                    nc.vector.tensor_mul(
                                        et[:, 0:128], et[:, 0:128], utri[:])
                                nc.tensor.matmul(
                                    po[:, q_rel:512], v_b[:, j, :],
                                    et[:, :N],
                                    start=(j == 0), stop=(j == nj - 1))
                            # evacuate numerators (undivided) + denominator
                            r = (hp * 2 + h01) * 2 + c
                            g0 = (b * 2 + c) * 512
                            dst = aoT[hp][hbase:hbase + 64, g0:g0 + 512]
                            nc.vector.tensor_copy(dst, po[0:64, :])
                            dn = dnp.tile([1, 512], F32, tag="dn")
                            nc.scalar.copy(dn[:], po[64:65, :])
                            nc.sync.dma_start(den_pack[r:r + 1, :], dn[:])
                            div_work.append((hp, hbase, r, g0))
                # per-batch softmax normalization: 1/den, bcast, multiply
                nc.vector.reciprocal(rec_pack[:], den_pack[:])
                for hp, hbase, r, g0 in div_work:
                    rc0 = dnp.tile([1, 512], F32, tag="rc0")
                    nc.sync.dma_start(rc0[:], rec_pack[r:r + 1, :])
                    bc = bcp.tile([128, 512], F32, tag="bc")
                    nc.gpsimd.partition_broadcast(bc[:], rc0[:])
                    dst = aoT[hp][hbase:hbase + 64, g0:g0 + 512]
                    nc.vector.tensor_mul(dst, dst, bc[hbase:hbase + 64, :])

            # AllToAll: re-shard from heads to tokens
            in_b = dram.tile([N_CORES, 2 * 128, 512], BF16, tag="a2a_in")
            out_b = dram.tile([N_CORES, 2 * 128, 512], BF16, tag="a2a_out")
            for j in range(N_CORES):
                for hp in range(2):
                    nc.sync.dma_start(
                        in_b[j, hp * 128:hp * 128 + 128, :],
                        aoT[hp][:, j * 512:j * 512 + 512])
            nc.gpsimd.collective_compute(
                "AllToAll", mybir.AluOpType.bypass,
                replica_groups=[list(range(N_CORES))],
                ins=[in_b.opt()], outs=[out_b.opt()])
            aog = gp.tile([128, NKO, 512], BF16, tag="aog")
            nc.sync.dma_start(
                aog[:], out_b.rearrange("s (h p) q -> p (s h) q", p=128))

            # output projection: out[t, e] = sum_c aog[c, t] wo[c, e]
            for n in range(4):
                wo_sb = wop.tile([128, NKO, 512], BF16, tag="wo")
                nc.sync.dma_start(wo_sb[:], wo_r[:, :, n * 512:n * 512 + 512])
                for m in range(4):
                    ps = pp.tile([128, 512], F32, tag="mm")
                    for ko in range(NKO):
                        nc.tensor.matmul(
                            ps[:], aog[:, ko, m * 128:m * 128 + 128],
                            wo_sb[:, ko, :],
                            start=(ko == 0), stop=(ko == NKO - 1))
                    ot = op.tile([128, 512], F32, tag="ot")
                    nc.any.tensor_copy(ot[:], ps[:])
                    nc.sync.dma_start(
                        out.ap()[m * 128:m * 128 + 128,
                                 n * 512:n * 512 + 512], ot[:])

    nc.compile()
    return nc


def _prep_inputs(x, wq, wk, wv, wo, cos, sin):
    bf = ml_dtypes.bfloat16
    x2 = np.ascontiguousarray(x.reshape(NT, D).T).astype(bf)  # [D, NT]
    # de-interleave rope pairs within each head: col j -> (j%2)*32 + j//2
    wq_p = wq.reshape(D, QH, 32, 2).transpose(0, 1, 3, 2).reshape(D, QH * HD)
    wk_p = wk.reshape(D, KVH, 32, 2).transpose(0, 1, 3, 2).reshape(D, KVH * HD)
    cosT = np.ascontiguousarray(cos.T)  # [32, T]
    sinT = np.ascontiguousarray(sin.T)
    ctile = np.concatenate([cosT] * 4, axis=0).astype(bf)
    stile = np.concatenate([sinT, -sinT, sinT, -sinT], axis=0).astype(bf)
    wo_b = wo.astype(bf)
    in_maps = []
    for c in range(N_CORES):
        wq_s = np.ascontiguousarray(wq_p[:, c * 256:(c + 1) * 256]).astype(bf)
        wkv_s = np.concatenate(
            [wk_p[:, c * 64:(c + 1) * 64], wv[:, c * 64:(c + 1) * 64]],
            axis=1).astype(bf)
        in_maps.append({
            "xT": x2, "wq": wq_s, "wkv": wkv_s, "wo": wo_b,
            "ctile": ctile, "stile": stile,
        })
    return in_maps


def _run(inputs, trace=False):
    if "nc" not in _CACHE:
        _CACHE["nc"] = _build()
    nc = _CACHE["nc"]
    in_maps = _prep_inputs(
        np.asarray(inputs["x"], dtype=np.float32),
        np.asarray(inputs["wq"], dtype=np.float32),
        np.asarray(inputs["wk"], dtype=np.float32),
        np.asarray(inputs["wv"], dtype=np.float32),
        np.asarray(inputs["wo"], dtype=np.float32),
        np.asarray(inputs["cos"], dtype=np.float32),
        np.asarray(inputs["sin"], dtype=np.float32),
    )
    res = run_bass_kernel_spmd(nc, in_maps, core_ids=list(range(N_CORES)),
                               trace=trace)
    full = np.empty((NT, D), dtype=np.float32)
    for c in range(N_CORES):
        full[c * ROWS:(c + 1) * ROWS] = res.results[c]["out"]
    return full.reshape(B, T, D), res


def kernel(**inputs) -> np.ndarray:
    out, _ = _run(inputs, trace=False)
    return out


def kernel_traced(**inputs):
    out, res = _run(inputs, trace=True)
    return out, res


# revision 7
# speedup vs baseline: 1.1117x; 1.1117x over previous
"""GQA attention forward (B=4, T=1024, D=2048, 32 q-heads / 8 kv-heads, RoPE,
causal) distributed over 8 TRN2 NeuronCores.

Sharding: head-parallel tensor parallelism. Core c owns q-heads 4c..4c+3 and
kv-head c (wq/wk/wv column shards). Attention output (still sharded by head,
transposed layout [head_dim, tokens]) is re-sharded to token-parallel via one
AllToAll (2 MB/rank, bf16); each core then computes its 512-token row slice of
the output projection against the full wo.

Device layouts (per core):
  xT   [2048, 4096]  bf16  - x transposed, tokens batch-major
  qT   [128, 4096]x2 bf16  - 2 heads per tile, RoPE'd; head-dim de-interleaved
  kT2  [128, 4096]   bf16  - kv-head kT duplicated in both 64-partition halves
  vT   [64, 4096]    bf16  - PE-transposed per 128-token tile into v[128, 65]
                             with a ones column (softmax denominator trick)
  scores sT[k, q] in PSUM -> exp on ACT (scale=1/8 folded) -> bf16
  attn@v: lhsT = v_aug [128, 65], rhs = expT -> psum [65, 512] accumulated
  divide by denominator row via reciprocal + gpsimd partition_broadcast
  A2A -> aoT_g [2048(c), 512(t)] -> out[t, e] = sum_c aoT_g[c, t] * wo[c, e]

RoPE with de-interleaved head dims ([32 reals; 32 imags] per 64-row head):
  out = x*C + shift32(x*S), C = [c;c;...], S = [s;-s;s;-s] (host-built tiles).
"""

import sys

if "/opt/trn_rl_repo" not in sys.path:
    sys.path.insert(0, "/opt/trn_rl_repo")

import numpy as np
import ml_dtypes

import concourse.bass as bass
import concourse.mybir as mybir
import concourse.tile as tile
from concourse import bacc
from concourse.bass_utils import run_bass_kernel_spmd
from concourse.masks import make_identity, make_upper_triangular

BF16 = mybir.dt.bfloat16
F32 = mybir.dt.float32

B, T, D = 4, 1024, 2048
QH, KVH, HD = 32, 8, 64
N_CORES = 8
NT = B * T            # 4096 global tokens
NKO = D // 128        # 16 contraction subtiles
ROWS = NT // N_CORES  # 512 output rows per core
HPC = QH // N_CORES   # 4 q heads per core

_CACHE = {}


def _build():
    nc = bacc.Bacc("TRN2", target_bir_lowering=False, debug=False,
                   num_devices=N_CORES)

    xT = nc.dram_tensor("xT", [D, NT], BF16, kind="ExternalInput")
    wq = nc.dram_tensor("wq", [D, HPC * HD], BF16, kind="ExternalInput")
    wkv = nc.dram_tensor("wkv", [D, 2 * HD], BF16, kind="ExternalInput")
    wo = nc.dram_tensor("wo", [D, D], BF16, kind="ExternalInput")
    ct = nc.dram_tensor("ctile", [128, T], BF16, kind="ExternalInput")
    st = nc.dram_tensor("stile", [128, T], BF16, kind="ExternalInput")
    out = nc.dram_tensor("out", [ROWS, D], F32, kind="ExternalOutput")

    xT_r = xT.ap().rearrange("(ko p) t -> p ko t", p=128)
    wq_r = wq.ap().rearrange("(ko p) m -> p ko m", p=128)
    wkv_r = wkv.ap().rearrange("(ko p) m -> p ko m", p=128)
    wo_r = wo.ap().rearrange("(ko p) e -> p ko e", p=128)

    with tile.TileContext(nc) as tc:
        import contextlib
        with contextlib.ExitStack() as ctx:
            const = ctx.enter_context(tc.tile_pool(name="const", bufs=1))
            xp = ctx.enter_context(tc.tile_pool(name="xp", bufs=3))
            big = ctx.enter_context(tc.tile_pool(name="big", bufs=1))
            vp = ctx.enter_context(tc.tile_pool(name="vp", bufs=2))
            ep = ctx.enter_context(tc.tile_pool(name="ep", bufs=4))
            xsp = ctx.enter_context(tc.tile_pool(name="xsp", bufs=2))
            dnp = ctx.enter_context(tc.tile_pool(name="dnp", bufs=3))
            bcp = ctx.enter_context(tc.tile_pool(name="bcp", bufs=2))
            wop = ctx.enter_context(tc.tile_pool(name="wop", bufs=2))
            gp = ctx.enter_context(tc.tile_pool(name="gp", bufs=1))
            op = ctx.enter_context(tc.tile_pool(name="op", bufs=2))
            dram = ctx.enter_context(tc.tile_pool(name="dram", bufs=1,
                                                  space="DRAM"))
            pp = ctx.enter_context(tc.tile_pool(name="pp", bufs=2,
                                                space="PSUM"))
            sp = ctx.enter_context(tc.tile_pool(name="sp", bufs=3,
                                                space="PSUM"))
            ap = ctx.enter_context(tc.tile_pool(name="ap", bufs=3,
                                                space="PSUM"))

            # constants / weights
            wq_sb = const.tile([128, NKO, HPC * HD], BF16, tag="wq")
            nc.sync.dma_start(wq_sb[:], wq_r)
            wkv_sb = const.tile([128, NKO, 2 * HD], BF16, tag="wkv")
            nc.sync.dma_start(wkv_sb[:], wkv_r)
            ct_sb = const.tile([128, T], BF16, tag="ct")
            nc.sync.dma_start(ct_sb[:], ct.ap())
            st_sb = const.tile([128, T], BF16, tag="st")
            nc.sync.dma_start(st_sb[:], st.ap())
            utri = const.tile([128, 128], BF16, tag="utri")
            make_upper_triangular(nc, utri[:], val=1.0, diag=True)
            ident = const.tile([64, 64], BF16, tag="ident")
            make_identity(nc, ident[:])

            qT = [big.tile([128, NT], BF16, tag=f"qT{hp}", name=f"qT{hp}")
                  for hp in range(2)]
            kT2 = big.tile([128, NT], BF16, tag="kT2")
            vT = big.tile([64, NT], BF16, tag="vT")
            aoT = [big.tile([128, NT], BF16, tag=f"aoT{hp}", name=f"aoT{hp}")
                   for hp in range(2)]

            def rope(dst, xs, xs2, ps, rows, cs_sl, ss_sl):
                # dst = ps*C + shift32(ps*S) over `rows` partitions (64 or 128)
                nc.vector.scalar_tensor_tensor(
                    dst, ps[0:rows], 1.0, cs_sl[0:rows],
                    mybir.AluOpType.mult, mybir.AluOpType.mult)
                nc.vector.scalar_tensor_tensor(
                    xs[0:rows], ps[0:rows], 1.0, ss_sl[0:rows],
                    mybir.AluOpType.mult, mybir.AluOpType.mult)
                # shift-by-32 within each 64-row half (cross-partition: gpsimd)
                for g in range(rows // 32):
                    a, b_ = g * 32, (g ^ 1) * 32
                    nc.vector.tensor_copy(xs2[a:a + 32], xs[b_:b_ + 32])
                nc.vector.tensor_add(dst, dst, xs2[0:rows])

            in_b = dram.tile([N_CORES, 2 * 128, 512], BF16, tag="a2a_in")
            out_b = dram.tile([N_CORES, 2 * 128, 512], BF16, tag="a2a_out")

            for b in range(B):
                den_pack = dnp.tile([8, 512], F32, tag="den")
                rec_pack = dnp.tile([8, 512], F32, tag="rec")
                xc = []
                for half in range(2):
                    t0 = b * T + half * 512
                    xt = xp.tile([128, NKO, 512], BF16, tag="x")
                    nc.sync.dma_start(xt[:], xT_r[:, :, t0:t0 + 512])
                    xc.append(xt)

                # q projection + rope
                for hp in range(2):
                    for half in range(2):
                        t0 = b * T + half * 512
                        ps = pp.tile([128, 512], F32, tag="mm")
                        for ko in range(NKO):
                            nc.tensor.matmul(
                                ps[:], wq_sb[:, ko, hp * 128:hp * 128 + 128],
                                xc[half][:, ko, :],
                                start=(ko == 0), stop=(ko == NKO - 1))
                        xs = xsp.tile([128, 512], BF16, tag="xs")
                        xs2 = xsp.tile([128, 512], BF16, tag="xs2")
                        csl = ct_sb[:, half * 512:half * 512 + 512]
                        ssl = st_sb[:, half * 512:half * 512 + 512]
                        rope(qT[hp][:, t0:t0 + 512], xs, xs2, ps, 128,
                             csl, ssl)

                # kv projection: rope k into both halves of kT2; evac vT
                for half in range(2):
                    t0 = b * T + half * 512
                    ps = pp.tile([128, 512], F32, tag="mm")
                    for ko in range(NKO):
                        nc.tensor.matmul(
                            ps[:], wkv_sb[:, ko, :], xc[half][:, ko, :],
                            start=(ko == 0), stop=(ko == NKO - 1))
                    xs = xsp.tile([128, 512], BF16, tag="xs")
                    xs2 = xsp.tile([128, 512], BF16, tag="xs2")
                    csl = ct_sb[:, half * 512:half * 512 + 512]
                    ssl = st_sb[:, half * 512:half * 512 + 512]
                    rope(kT2[0:64, t0:t0 + 512], xs, xs2, ps, 64, csl, ssl)
                    nc.vector.tensor_copy(kT2[64:128, t0:t0 + 512],
                                          kT2[0:64, t0:t0 + 512])
                    nc.vector.tensor_copy(vT[:, t0:t0 + 512], ps[64:128, :])

                # transpose v into natural layout + ones column
                v_b = vp.tile([128, 8, HD + 1], BF16, tag="v")
                for j in range(8):
                    ks = b * T + j * 128
                    tps = pp.tile([128, 64], BF16, tag="mm")
                    nc.tensor.transpose(tps[:], vT[:, ks:ks + 128], ident[:])
                    nc.vector.tensor_copy(v_b[:, j, 0:HD], tps[:])
                nc.any.memset(v_b[:, :, HD:HD + 1], 1.0)

                # attention for this batch: 4 heads x 2 q-chunks of 512
                # the two heads of a pair are interleaved per k-tile so their
                # K=64 score matmuls occupy disjoint PE row groups (0:64 and
                # 64:128) back-to-back -> concurrent in the array
                div_work = []
                for hp in range(2):
                    for c in range(2):
                        po2 = [ap.tile([65, 512], F32, tag="attn",
                                       name=f"po{h01}") for h01 in range(2)]
                        nj = 4 * c + 4
                        for j in range(nj):
                            q_lo = max(c * 512, j * 128)
                            N = (c + 1) * 512 - q_lo
                            q_rel = q_lo - c * 512
                            ks = b * T + j * 128
                            ets = []
                            for h01 in range(2):
                                hbase = h01 * 64
                                ps = sp.tile([128, 512], F32, tag="score",
                                             name=f"ps{h01}")
                                nc.tensor.matmul(
                                    ps[:, :N],
                                    kT2[hbase:hbase + 64, ks:ks + 128],
                                    qT[hp][hbase:hbase + 64,
                                           b * T + q_lo:b * T + q_lo + N],
                                    start=True, stop=True)
                                et = ep.tile([128, 512], BF16, tag="et",
                                             name=f"et{h01}")
                                nc.scalar.activation(
                                    et[:, :N], ps[:, :N],
                                    mybir.ActivationFunctionType.Exp,
                                    scale=0.125)
                                if j >= 4 * c:
                                    nc.vector.tensor_mul(
                                        et[:, 0:128], et[:, 0:128], utri[:])
                                ets.append(et)
                            for h01 in range(2):
                                nc.tensor.matmul(
                                    po2[h01][:, q_rel:512], v_b[:, j, :],
                                    ets[h01][:, :N],
                                    start=(j == 0), stop=(j == nj - 1))
                        for h01 in range(2):
                            hbase = h01 * 64
                            po = po2[h01]
                            # evacuate numerators (undivided) + denominator
                            r = (hp * 2 + h01) * 2 + c
                            g0 = (b * 2 + c) * 512
                            dst = aoT[hp][hbase:hbase + 64, g0:g0 + 512]
                            nc.vector.tensor_copy(dst, po[0:64, :])
                            dn = dnp.tile([1, 512], F32, tag="dn")
                            nc.scalar.copy(dn[:], po[64:65, :])
                            nc.sync.dma_start(den_pack[r:r + 1, :], dn[:])
                            div_work.append((hp, hbase, r, g0))
                # per-batch softmax normalization: 1/den, bcast, multiply
                nc.vector.reciprocal(rec_pack[:], den_pack[:])
                for hp, hbase, r, g0 in div_work:
                    rc0 = dnp.tile([1, 512], F32, tag="rc0")
                    nc.sync.dma_start(rc0[:], rec_pack[r:r + 1, :])
                    bc = bcp.tile([128, 512], F32, tag="bc")
                    nc.gpsimd.partition_broadcast(bc[:], rc0[:])
                    dst = aoT[hp][hbase:hbase + 64, g0:g0 + 512]
                    nc.vector.tensor_mul(dst, dst, bc[hbase:hbase + 64, :])
                # stream this batch's A2A input blocks out as soon as divided
                for c in range(2):
                    jslot = b * 2 + c
                    for hp in range(2):
                        nc.sync.dma_start(
                            in_b[jslot, hp * 128:hp * 128 + 128, :],
                            aoT[hp][:, jslot * 512:jslot * 512 + 512])

            # AllToAll: re-shard from heads to tokens
            nc.gpsimd.collective_compute(
                "AllToAll", mybir.AluOpType.bypass,
                replica_groups=[list(range(N_CORES))],
                ins=[in_b.opt()], outs=[out_b.opt()])
            aog = gp.tile([128, NKO, 512], BF16, tag="aog")
            nc.sync.dma_start(
                aog[:], out_b.rearrange("s (h p) q -> p (s h) q", p=128))

            # output projection: out[t, e] = sum_c aog[c, t] wo[c, e]
            for n in range(4):
                wo_sb = wop.tile([128, NKO, 512], BF16, tag="wo")
                nc.sync.dma_start(wo_sb[:], wo_r[:, :, n * 512:n * 512 + 512])
                for m in range(4):
                    ps = pp.tile([128, 512], F32, tag="mm")
                    for ko in range(NKO):
                        nc.tensor.matmul(
                            ps[:], aog[:, ko, m * 128:m * 128 + 128],
                            wo_sb[:, ko, :],
                            start=(ko == 0), stop=(ko == NKO - 1))
                    ot = op.tile([128, 512], F32, tag="ot")
                    nc.vector.tensor_copy(ot[:], ps[:])
                    nc.sync.dma_start(
                        out.ap()[m * 128:m * 128 + 128,
                                 n * 512:n * 512 + 512], ot[:])

    nc.compile()
    return nc


def _prep_inputs(x, wq, wk, wv, wo, cos, sin):
    bf = ml_dtypes.bfloat16
    x2 = np.ascontiguousarray(x.reshape(NT, D).T).astype(bf)  # [D, NT]
    # de-interleave rope pairs within each head: col j -> (j%2)*32 + j//2
    wq_p = wq.reshape(D, QH, 32, 2).transpose(0, 1, 3, 2).reshape(D, QH * HD)
    wk_p = wk.reshape(D, KVH, 32, 2).transpose(0, 1, 3, 2).reshape(D, KVH * HD)
    cosT = np.ascontiguousarray(cos.T)  # [32, T]
    sinT = np.ascontiguousarray(sin.T)
    ctile = np.concatenate([cosT] * 4, axis=0).astype(bf)
    stile = np.concatenate([sinT, -sinT, sinT, -sinT], axis=0).astype(bf)
    wo_b = wo.astype(bf)
    in_maps = []
    for c in range(N_CORES):
        wq_s = np.ascontiguousarray(wq_p[:, c * 256:(c + 1) * 256]).astype(bf)
        wkv_s = np.concatenate(
            [wk_p[:, c * 64:(c + 1) * 64], wv[:, c * 64:(c + 1) * 64]],
            axis=1).astype(bf)
        in_maps.append({
            "xT": x2, "wq": wq_s, "wkv": wkv_s, "wo": wo_b,
            "ctile": ctile, "stile": stile,
        })
    return in_maps


def _run(inputs, trace=False):
    if "nc" not in _CACHE:
        _CACHE["nc"] = _build()
    nc = _CACHE["nc"]
    in_maps = _prep_inputs(
        np.asarray(inputs["x"], dtype=np.float32),
        np.asarray(inputs["wq"], dtype=np.float32),
        np.asarray(inputs["wk"], dtype=np.float32),
        np.asarray(inputs["wv"], dtype=np.float32),
        np.asarray(inputs["wo"], dtype=np.float32),
        np.asarray(inputs["cos"], dtype=np.float32),
        np.asarray(inputs["sin"], dtype=np.float32),
    )
    res = run_bass_kernel_spmd(nc, in_maps, core_ids=list(range(N_CORES)),
                               trace=trace)
    full = np.empty((NT, D), dtype=np.float32)
    for c in range(N_CORES):
        full[c * ROWS:(c + 1) * ROWS] = res.results[c]["out"]
    return full.reshape(B, T, D), res


def kernel(**inputs) -> np.ndarray:
    out, _ = _run(inputs, trace=False)
    return out


def kernel_traced(**inputs):
    out, res = _run(inputs, trace=True)
    return out, res
